# revision 38
# baseline (speedup 1.0000x reference)
"""Windowed local self-attention (CrossAttention module with the context-
overwrite bug faithfully reproduced) on 8 Trainium2 NeuronCores.

Full-input contract: kernel(**inputs) takes the unsharded tensors and
returns the full (4, 4096, 1024) output. Internally the 64 independent
windows of 256 tokens are data-parallel sharded 8-per-core; the four
projection weights are broadcast to every core. No collectives needed.

All matmul operands are bf16 (host-cast): 1 cycle/row on the PE, half
the SBUF/DMA traffic of fp32, far less PE power than fp32 HIGH mode
(which triggered 50% periodic throttling in the fp32r version). PSUM
accumulation, softmax normalization and the final output stay fp32.

Key structure:
- X is transposed on the HOST: the kernel DMAs X^T tiles straight into
  SBUF, so no PE transposes / identity preamble at all.
- Windows processed in PAIRS (512 tokens): every projection/output
  matmul streams the max 512 moving rows, hiding LDWEIGHTS.
- V is stored interleaved per head as [v_h (64) | ones (64)]; the AV
  matmul then emits the attention numerator on rows 0-63 AND the
  softmax denominator (replicated) on rows 64-127 -- no row-sum matmul.
- The AV results of a head pair share one PSUM bank -> one reciprocal
  per two heads.
- Software pipelining: the attention phase of pair p is DVE/ACT-paced,
  so the projection chains of pair p+1 (and pair p's output-projection
  chains) are interleaved into its step loop to keep the PE streaming.

Per-core pipeline (window = 256 tokens, H=16 heads, DH=64):
  qT = Wq.T @ X.T   (lhsT=Wq tiles,  rhs=XT)          [o, i]
  kT = Wk.T @ X.T                                      [o, i]
  v  = X @ Wv       (lhsT=XT tiles,  rhs=Wv)           [j, v|1]
  per (window, head):
    simT = kT_h.T-free @ qT_h   -> [j, i] in PSUM     (j on partitions)
    es   = exp(0.125 * simT)    (ACT, PSUM->SBUF, bf16)
    av   = [v_h|1].T-free @ es  -> [128, i] PSUM
    rS   = 1/S   (one DVE reciprocal per head pair, full PSUM bank)
    o2T  = o2u * rS             (DVE, bf16 [o, i] SBUF)
  Y = o2T.T @ Wo       (lhsT=o2T tiles, rhs=Wo; zero bias added host-side)
"""

import numpy as np
import ml_dtypes

import concourse.bass as bass
import concourse.mybir as mybir
import concourse.tile as tile
from concourse import bacc, bass_utils
from concourse.bass_interp import get_hw_module

H = 16
DH = 64
WIN = 256
D = 1024
B = 4
N = 4096
N_CORES = 8
N_WIN_TOTAL = B * N // WIN          # 64
N_WIN = N_WIN_TOTAL // N_CORES      # 8 windows per core
TOK = N_WIN * WIN                   # 2048 token rows per core
PAIR = 2 * WIN                      # 512 tokens per window pair
SCALE = DH ** -0.5

F32 = mybir.dt.float32
BF16 = mybir.dt.bfloat16


def _body(tc, xqT, wq, wk, wv, wo, out, n_win):
    nc = tc.nc
    from contextlib import ExitStack

    n_pair = n_win // 2

    with ExitStack() as ctx:
        singles = ctx.enter_context(tc.tile_pool(name="singles", bufs=1))
        acts = ctx.enter_context(tc.tile_pool(name="acts", bufs=1))
        heads = ctx.enter_context(tc.tile_pool(name="heads", bufs=3))
        ypool = ctx.enter_context(tc.tile_pool(name="ypool", bufs=2))
        psA = ctx.enter_context(tc.tile_pool(name="psA", bufs=2, space="PSUM"))
        psS = ctx.enter_context(tc.tile_pool(name="psS", bufs=4, space="PSUM"))
        psV = ctx.enter_context(tc.tile_pool(name="psV", bufs=2, space="PSUM"))

        def emit_xt_dma(wp):
            halves = make_xt(wp)
            for hf in range(2):
                dma_xt_half(halves[hf], wp, hf)
            return halves

        # weights in half-tiles; DRAM side is host-prepacked so every DMA is
        # per-partition contiguous (128 descriptors of 8KB) -- the strided
        # rearrange descriptors cost 0.6-5.9us PER dma_start on the sync
        # engine and capped DMA at ~300GB/s in the v1 trace
        wsb = {}
        wdram = {"wq": wq, "wk": wk, "wv": wv, "wo": wo}
        for name in ("wq", "wk", "wv", "wo"):
            wsb[name] = [
                singles.tile([128, 4 * D], BF16, tag=f"{name}{hf}",
                             name=f"sb_{name}_{hf}")
                for hf in range(2)
            ]

        def dma_w_half(name, hf):
            nc.sync.dma_start(
                wsb[name][hf][:],
                wdram[name][:, hf * 4 * D:(hf + 1) * 4 * D],
            )

        def dma_w_chunk(name, hf, klp):
            nc.sync.dma_start(
                wsb[name][hf][:, klp * 2 * D:(klp + 1) * 2 * D],
                wdram[name][:, hf * 4 * D + klp * 2 * D:
                            hf * 4 * D + (klp + 1) * 2 * D],
            )

        def make_xt(wp):
            return [acts.tile([128, 4 * 512], BF16, tag=f"xt{hf}", bufs=2,
                              name=f"xt_{wp}_{hf}") for hf in range(2)]

        def dma_xt_half(t, wp, hf):
            nc.sync.dma_start(
                t[:],
                xqT[:, (wp * 2 + hf) * 2048:(wp * 2 + hf + 1) * 2048],
            )

        def dma_xt_chunk(t, wp, hf, klp):
            nc.sync.dma_start(
                t[:, klp * 1024:(klp + 1) * 1024],
                xqT[:, (wp * 2 + hf) * 2048 + klp * 1024:
                    (wp * 2 + hf) * 2048 + (klp + 1) * 1024],
            )

        # PE warm-up: the HAM clock gate needs ~3.4us of sustained activity
        # to lift the cold 1.2GHz throttle, and real work can't start until
        # DMA delivers data -- so stream dummy matmuls over a zeroed scratch
        # tile while the first chunks arrive, so real chains run warm
        scratch = singles.tile([128, 512], BF16, name="warm_scratch")
        nc.gpsimd.memset(scratch[:], 0.0)
        warm_ps = psV.tile([128, 512], F32, tag="av", name="warm_ps")
        for _ in range(10):
            nc.tensor.matmul(warm_ps[:], scratch[:, 0:128], scratch[:],
                             start=True, stop=True)

        # pair 0's gating transfers stream in chunks, interleaved so each
        # (xt, wq) chunk pair enables the next kl-slices of q matmuls
        xt0 = make_xt(0)
        for klp in range(2):
            dma_xt_chunk(xt0[0], 0, 0, klp)
            dma_w_chunk("wq", 0, klp)
        for klp in range(2):
            dma_xt_chunk(xt0[1], 0, 1, klp)
            dma_w_chunk("wq", 1, klp)
        for name in ("wk", "wv", "wo"):
            for hf in range(2):
                dma_w_half(name, hf)

        # v buffers: pair parity x window -> 4 buffers; per-head layout
        # [v_h (64 cols) | ones (64 cols)] so AV' yields sums on rows 64+.
        v2b = []
        for i in range(4):
            t = singles.tile([128, 2 * H * 128], BF16, name=f"v2_{i}")
            ones_view = t[:].rearrange("p (j h c) -> p j h c", j=2, h=H)[:, :, :, :DH]
            nc.gpsimd.memset(ones_view, 1.0)
            v2b.append(t)

        def proj_chains(wp, xt):
            """qT/kT/v chains for pair wp as a list of zero-arg closures."""
            proj = {}
            for pname in ("qT", "kT"):
                proj[pname] = acts.tile([128, 8 * 512], BF16, tag=pname,
                                        bufs=2, name=f"{pname}_{wp}")
            chains = []
            for ot in range(8):
                for pname, wname in (("qT", "wq"), ("kT", "wk")):
                    def qk_chain(ot=ot, pname=pname, wname=wname):
                        pq = psA.tile([128, 512], F32, tag="acc",
                                      name=f"pq_{wp}_{pname}_{ot}")
                        for kt in range(8):
                            hf, kl = kt // 4, kt % 4
                            nc.tensor.matmul(
                                pq[:],
                                wsb[wname][hf][:, kl * D + ot * 128:
                                               kl * D + (ot + 1) * 128],
                                xt[hf][:, kl * 512:(kl + 1) * 512],
                                start=(kt == 0),
                                stop=(kt == 7),
                            )
                        cp = (nc.vector.tensor_copy if pname == "qT"
                              else nc.scalar.copy)
                        cp(proj[pname][:, ot * 512:(ot + 1) * 512], pq[:])
                    chains.append(qk_chain)
            for tt in range(4):
                for oc in range(2):
                    def v_chain(tt=tt, oc=oc):
                        wl, jt = tt // 2, tt % 2
                        pv = psA.tile([128, 512], F32, tag="acc",
                                      name=f"pv_{wp}_{tt}_{oc}")
                        for kt in range(8):
                            hf, kl = kt // 4, kt % 4
                            nc.tensor.matmul(
                                pv[:],
                                xt[hf][:, kl * 512 + tt * 128:
                                       kl * 512 + (tt + 1) * 128],
                                wsb["wv"][hf][:, kl * D + oc * 512:
                                              kl * D + (oc + 1) * 512],
                                start=(kt == 0),
                                stop=(kt == 7),
                            )
                        vdst = v2b[(wp % 2) * 2 + wl]
                        dsl = vdst[:, jt * H * 128 + oc * 8 * 128:
                                   jt * H * 128 + (oc + 1) * 8 * 128]
                        nc.scalar.copy(
                            dsl.rearrange("p (h c) -> p h c", h=8)[:, :, DH:],
                            pv[:],
                        )
                    chains.append(v_chain)
            return proj, chains

        y_sb_t = {}

        def emit_y_group(wp, o2T, it, ec, pool=None, tag="acc"):
            # both ec halves share one y_sb [128, 1024]; the ec=1 group
            # flushes it with a single contiguous 2KB-per-partition DMA
            row0 = wp * PAIR
            pool = pool or psA
            kw = {"bufs": 4} if tag == "sim" else {}
            py = pool.tile([128, 512], F32, tag=tag,
                           name=f"py_{wp}_{it}_{ec}", **kw)
            for kt2 in range(8):
                hf, kl = kt2 // 4, kt2 % 4
                nc.tensor.matmul(
                    py[:],
                    o2T[:, kt2 * 512 + it * 128:kt2 * 512 + (it + 1) * 128],
                    wsb["wo"][hf][:, kl * D + ec * 512:kl * D + (ec + 1) * 512],
                    start=(kt2 == 0),
                    stop=(kt2 == 7),
                )
            if ec == 0:
                y_sb_t[it] = ypool.tile([128, 1024], BF16, tag="y",
                                        name=f"y_{wp}_{it}")
            y_sb = y_sb_t[it]
            cp = nc.vector.tensor_copy if ec == 0 else nc.scalar.copy
            cp(y_sb[:, ec * 512:(ec + 1) * 512], py[:])
            if ec == 1:
                nc.sync.dma_start(
                    out[row0 + it * 128:row0 + (it + 1) * 128, :],
                    y_sb_t.pop(it)[:],
                )

        deferred_y = []

        def attention(wp, qT, kT, o2T, extra, last=False,
                      defer_trailing=False):
            """16 groups of paired heads (2m, 2m+1) per pair; `extra` chains
            are paced through the loop to keep the PE streaming while DVE/ACT
            normalize. The paired heads' qT/kT rows live on complementary
            partition halves, so their sim matmuls issue adjacently as
            64x128 row-tiles T(0,0)/T(64,0) and stream CONCURRENTLY --
            halving sim cost vs serial emission. Head h=2m+s maps to o2T
            block s*4+m//2, rows 64*(m%2) (Wo rows are host-permuted to
            match), so the pair's two normalize-muls still merge into one
            stride-4 two-block DVE op."""
            G = 16
            es_t = {}
            av_t = {}

            def emit_sim(g):
                wl, m = g // 8, g % 8
                ocol = m * 512 + wl * WIN
                # one two-bank PSUM tile per group: both heads' sims land in
                # [128, 1024] and ONE exp activation covers them (the ~260ns
                # per-op ACT overhead amortizes over 2 heads)
                ps = psS.tile([128, 1024], F32, tag="sim", bufs=2,
                              name=f"sim_{wp}_{g}")
                for jt in range(2):
                    for s in range(2):
                        prow = s * 64
                        nc.tensor.matmul(
                            ps[:, s * 512 + jt * WIN:s * 512 + (jt + 1) * WIN],
                            kT[prow:prow + 64,
                               ocol + jt * 128:ocol + (jt + 1) * 128],
                            qT[prow:prow + 64, ocol:ocol + WIN],
                            start=True,
                            stop=True,
                        )
                e = heads.tile([128, 1024], BF16, tag="es", bufs=4,
                               name=f"es_{wp}_{g}")
                nc.scalar.activation(
                    e[:], ps[:], mybir.ActivationFunctionType.Exp,
                    scale=SCALE,
                )
                es_t[g] = e

            def emit_av(g):
                wl, m = g // 8, g % 8
                av2 = psV.tile([128, 512], F32, tag="av", name=f"av_{wp}_{g}")
                es = es_t.pop(g)
                for s in range(2):
                    h = 2 * m + s
                    for jt in range(2):
                        nc.tensor.matmul(
                            av2[:, s * WIN:(s + 1) * WIN],
                            v2b[(wp % 2) * 2 + wl][:, (jt * H + h) * 128:
                                                   (jt * H + h + 1) * 128],
                            es[:, s * 512 + jt * WIN:s * 512 + (jt + 1) * WIN],
                            start=(jt == 0),
                            stop=(jt == 1),
                        )
                av_t[g] = av2

            def emit_epilogue(g):
                wl, m = g // 8, g % 8
                av2 = av_t.pop(g)
                rs = heads.tile([128, 512], F32, tag="rs", name=f"rs_{wp}_{g}")
                # sums live on rows 0:64 (ones-first v2b layout) so the
                # custom-DVE reciprocal runs on a base-0 half tile
                nc.vector.reciprocal_approx_fast(rs[0:64, :], av2[0:64, :])
                r0 = 64 * (m % 2)
                dst = o2T[r0:r0 + 64, :] \
                    .rearrange("p (b c) -> p b c", c=512) \
                    [:, (m // 2)::4, wl * WIN:(wl + 1) * WIN]
                nc.vector.tensor_mul(
                    dst,
                    av2[64:128, :].rearrange("p (b c) -> p b c", b=2),
                    rs[0:64, :].rearrange("p (b c) -> p b c", b=2),
                )

            # last pair only: window-1 y groups (it=2) accumulate
            # incrementally as head blocks normalize, trimming the tail
            y_inc = {}

            def emit_y_inc(g):
                row0 = wp * PAIR
                b = (g - 9) // 2
                for ec in range(2):
                    if g == 9:
                        y_inc[ec] = psA.tile([128, 512], F32, tag="acc",
                                             name=f"pyi_{wp}_{ec}")
                    for kt2 in (b, b + 4):
                        hf, kl = kt2 // 4, kt2 % 4
                        nc.tensor.matmul(
                            y_inc[ec][:],
                            o2T[:, kt2 * 512 + 2 * 128:kt2 * 512 + 3 * 128],
                            wsb["wo"][hf][:, kl * D + ec * 512:
                                          kl * D + (ec + 1) * 512],
                            start=(g == 9 and kt2 == b),
                            stop=(g == 15 and kt2 == b + 4),
                        )

            def flush_y_inc():
                row0 = wp * PAIR
                y_sb = ypool.tile([128, 1024], BF16, tag="y",
                                  name=f"y_{wp}_2")
                for ec in range(2):
                    cp = nc.vector.tensor_copy if ec == 0 else nc.scalar.copy
                    cp(y_sb[:, ec * 512:(ec + 1) * 512], y_inc[ec][:])
                nc.sync.dma_start(out[row0 + 256:row0 + 384, :], y_sb[:])

            n_extra = len(extra)
            ch_i = 0
            # two groups of sims (then avs) per batch: halves the number of
            # 64x128 <-> 128x128 tiling-mode transitions on the PE array
            emit_sim(0)
            emit_sim(1)
            for g in range(0, G, 2):
                if g == 8:
                    # must precede y_inc(9): it reuses the same psA banks,
                    # and y_inc holds them with open groups through g15
                    emit_y_group(wp, o2T, 1, 0)
                    emit_y_group(wp, o2T, 1, 1)
                for gg in (g, g + 1):
                    emit_av(gg)
                    emit_epilogue(gg)
                    if last and gg in (9, 11, 13, 15):
                        emit_y_inc(gg)
                if g + 2 < G:
                    emit_sim(g + 2)
                    emit_sim(g + 3)
                if g == 6:
                    # window 0 fully normalized at group 7: flow its Y groups
                    emit_y_group(wp, o2T, 0, 0)
                    emit_y_group(wp, o2T, 0, 1)
                # last pair: its extras are psA-bank y chains that must all
                # retire before y_inc claims those banks at group 9
                pace = 10 if last else G
                while ch_i < n_extra and ch_i * pace < n_extra * (g + 2):
                    extra[ch_i]()
                    ch_i += 1
            if last:
                # both it3 chains run in an idle wide sim bank so the tail
                # never waits on psA (held by y_inc until its flush)
                row0 = wp * PAIR
                pw = psS.tile([128, 1024], F32, tag="sim", bufs=2,
                              name=f"py3_{wp}")
                for ec in range(2):
                    for kt2 in range(8):
                        hf, kl = kt2 // 4, kt2 % 4
                        nc.tensor.matmul(
                            pw[:, ec * 512:(ec + 1) * 512],
                            o2T[:, kt2 * 512 + 3 * 128:kt2 * 512 + 4 * 128],
                            wsb["wo"][hf][:, kl * D + ec * 512:
                                          kl * D + (ec + 1) * 512],
                            start=(kt2 == 0),
                            stop=(kt2 == 7),
                        )
                flush_y_inc()
                # split the final store so the ec0 half flies while the ec1
                # chain/copy finishes -- shrinks the post-last-matmul tail
                y_sb = ypool.tile([128, 1024], BF16, tag="y", name=f"y_{wp}_3")
                nc.vector.tensor_copy(y_sb[:, 0:512], pw[:, 0:512])
                nc.sync.dma_start(out[row0 + 384:row0 + 512, 0:512],
                                  y_sb[:, 0:512])
                nc.scalar.copy(y_sb[:, 512:1024], pw[:, 512:1024])
                nc.sync.dma_start(out[row0 + 384:row0 + 512, 512:1024],
                                  y_sb[:, 512:1024])
            elif defer_trailing:
                # hand this pair's trailing y chains to the LAST pair's
                # attention loop -- it has no projection chains to pace, so
                # these fill its dependency-latency gaps
                for it in range(2, 4):
                    for ec in range(2):
                        deferred_y.append(
                            lambda it=it, ec=ec, o2T=o2T, wp=wp:
                            emit_y_group(wp, o2T, it, ec))
            else:
                for it in range(2, 4):
                    emit_y_group(wp, o2T, it, 0)
                    emit_y_group(wp, o2T, it, 1)

        proj, chains0 = proj_chains(0, xt0)
        # pair 0 q chains: kl-major sweep holding all 8 PSUM banks (nothing
        # else uses PSUM this early) so the PE consumes each 256KB chunk of
        # wq/xt the moment its DMA lands
        pq0 = []
        pq0_wide = [psS.tile([128, 1024], F32, tag="sim", bufs=2,
                             name=f"pq0_w_{i}") for i in range(2)]
        for ot in range(8):
            if ot < 2:
                pq0.append(psA.tile([128, 512], F32, tag="acc",
                                    name=f"pq0_qT_{ot}"))
            elif ot < 6:
                w = pq0_wide[(ot - 2) // 2]
                pq0.append(w[:, ((ot - 2) % 2) * 512:((ot - 2) % 2 + 1) * 512])
            else:
                pq0.append(psV.tile([128, 512], F32, tag="av",
                                    name=f"pq0_qT_{ot}"))
        for hf in range(2):
            for kl in range(4):
                kt = hf * 4 + kl
                for ot in range(8):
                    nc.tensor.matmul(
                        pq0[ot][:],
                        wsb["wq"][hf][:, kl * D + ot * 128:
                                      kl * D + (ot + 1) * 128],
                        xt0[hf][:, kl * 512:(kl + 1) * 512],
                        start=(kt == 0),
                        stop=(kt == 7),
                    )
        for ot in range(8):
            # alternate engines so the eight copies don't serialize on DVE
            cp = nc.vector.tensor_copy if ot % 2 == 0 else nc.scalar.copy
            cp(proj["qT"][:, ot * 512:(ot + 1) * 512], pq0[ot][:])
        for ch in chains0[1::2][:8] + chains0[16:]:
            ch()
        held = []
        for wp in range(n_pair):
            o2T = acts.tile([128, 8 * 512], BF16, tag="o2T", bufs=2,
                            name=f"o2T_{wp}")
            if wp + 1 < n_pair:
                xt_next = emit_xt_dma(wp + 1)
                proj_next, chains_next = proj_chains(wp + 1, xt_next)
                if wp + 2 == n_pair and n_pair > 1:
                    # hold back the last pair's latest-needed chains (q7/k7
                    # feed sims for heads 14/15; the oc1 window-1 v chains
                    # feed avs from group 12) to fill its latency gaps
                    held = [chains_next[i] for i in (14, 15, 21, 23)]
                    chains_next = [c for i, c in enumerate(chains_next)
                                   if i not in (14, 15, 21, 23)]
            else:
                proj_next, chains_next = None, held + deferred_y
            attention(wp, proj["qT"], proj["kT"], o2T, chains_next,
                      last=(wp + 1 == n_pair),
                      defer_trailing=(wp + 2 == n_pair))
            proj = proj_next


_CACHE = {}


def _build(n_win=N_WIN):
    key = n_win
    if key in _CACHE:
        return _CACHE[key]
    tok = n_win * WIN
    nc = bacc.Bacc(
        "TRN2", target_bir_lowering=False, debug=False, num_devices=N_CORES
    )
    # host-prepacked layouts: every DMA source is per-partition contiguous
    xqT = nc.dram_tensor("xqT", [128, tok * 8], BF16, kind="ExternalInput").ap()
    wq = nc.dram_tensor("Wq", [128, 8 * D], BF16, kind="ExternalInput").ap()
    wk = nc.dram_tensor("Wk", [128, 8 * D], BF16, kind="ExternalInput").ap()
    wv = nc.dram_tensor("Wv", [128, 8 * D], BF16, kind="ExternalInput").ap()
    wo = nc.dram_tensor("Wo", [128, 8 * D], BF16, kind="ExternalInput").ap()
    out = nc.dram_tensor("out", [tok, D], BF16, kind="ExternalOutput").ap()
    with tile.TileContext(nc) as tc:
        _body(tc, xqT, wq, wk, wv, wo, out, n_win)
    nc.compile()
    nc.m = get_hw_module(nc.m)
    _CACHE[key] = nc
    return nc


def _pack_w(W):
    # wpre[p, hf*4096 + kt*1024 + c] = W[hf*512 + kt*128 + p, c]
    bf = ml_dtypes.bfloat16
    W = np.asarray(W, np.float32).astype(bf)
    return np.ascontiguousarray(
        W.reshape(2, 4, 128, D).transpose(2, 0, 1, 3).reshape(128, 8 * D)
    )


def _wo_perm():
    # o2T index of head h=2m+s, dim d is (s*4 + m//2)*128 + 64*(m%2) + d;
    # permute Wo rows so the y matmul contracts o2T directly
    perm = np.empty(D, np.int64)
    for h in range(H):
        m, s = h // 2, h % 2
        base = (s * 4 + m // 2) * 128 + 64 * (m % 2)
        perm[base:base + DH] = np.arange(h * DH, (h + 1) * DH)
    return perm


def _pack_x(q2, n_pair):
    # xpre[p, (wp*2+hf)*2048 + dt*512 + t] = X[wp*512 + t, hf*512 + dt*128 + p]
    return np.ascontiguousarray(
        q2.reshape(n_pair, 512, 2, 4, 128)
        .transpose(4, 0, 2, 3, 1)
        .reshape(128, n_pair * 4096)
    )


def run(query, Wq, Wk, Wv, Wo, bo, n_win=N_WIN, **spmd_kwargs):
    nc = _build(n_win)
    tok = n_win * WIN
    n_pair = n_win // 2
    bf = ml_dtypes.bfloat16
    q2 = np.asarray(query, dtype=np.float32).reshape(-1, D).astype(bf)
    weights = {
        "Wq": _pack_w(Wq),
        "Wk": _pack_w(Wk),
        "Wv": _pack_w(Wv),
        "Wo": _pack_w(np.asarray(Wo, np.float32)[_wo_perm()]),
    }
    in_maps = []
    for c in range(N_CORES):
        m = {"xqT": _pack_x(q2[c * TOK:c * TOK + tok], n_pair)}
        m.update(weights)
        in_maps.append(m)
    try:
        res = bass_utils.run_bass_kernel_spmd(
            nc, in_maps, core_ids=list(range(N_CORES)), **spmd_kwargs
        )
    except Exception:
        # transient NRT_EXEC_UNIT_UNRECOVERABLE wedges clear on retry
        res = bass_utils.run_bass_kernel_spmd(
            nc, in_maps, core_ids=list(range(N_CORES)), **spmd_kwargs
        )
    outs = [np.asarray(res.results[c]["out"]).astype(np.float32)
            for c in range(N_CORES)]
    return outs, res


def kernel(query, context, Wq, Wk, Wv, Wo, bo):
    outs, _ = run(query, Wq, Wk, Wv, Wo, bo)
    y = np.concatenate(outs, axis=0).reshape(B, N, D)
    bo = np.asarray(bo, np.float32)
    if bo.any():
        y = y + bo  # bias is structurally zero for this problem; host-add keeps exactness
    return y.astype(np.float32)



# revision 39
# speedup vs baseline: 1.0081x; 1.0081x over previous
"""Windowed local self-attention (CrossAttention module with the context-
overwrite bug faithfully reproduced) on 8 Trainium2 NeuronCores.

Full-input contract: kernel(**inputs) takes the unsharded tensors and
returns the full (4, 4096, 1024) output. Internally the 64 independent
windows of 256 tokens are data-parallel sharded 8-per-core; the four
projection weights are broadcast to every core. No collectives needed.

All matmul operands are bf16 (host-cast): 1 cycle/row on the PE, half
the SBUF/DMA traffic of fp32, far less PE power than fp32 HIGH mode
(which triggered 50% periodic throttling in the fp32r version). PSUM
accumulation, softmax normalization and the final output stay fp32.

Key structure:
- X and all weights are repacked on the HOST so every DMA source is
  per-partition contiguous (128 x 8KB descriptors; ~430 GB/s vs ~300
  for strided patterns) -- no PE transposes / identity preamble at all.
- Warm-up: dummy matmuls over a zeroed scratch tile lift the HAM clock
  throttle (1.2 -> 2.4 GHz needs ~3.4us sustained PE activity) while
  the first DMA chunks stream in; pair-0 q chains are kl-chunk-gated.
- Windows processed in PAIRS (512 tokens): every projection/output
  matmul streams the max 512 moving rows, hiding LDWEIGHTS.
- Heads paired (2m, 2m+1) live on complementary qT/kT partition halves:
  their sim matmuls issue adjacently as 64x128 row-tiles T(0,0)/T(64,0)
  and stream CONCURRENTLY (2x sim throughput). Two groups of sims/avs
  are batched per stretch to halve tiling-mode-switch bubbles.
- V is stored interleaved per head as [ones (64) | v_h (64)]; the AV
  matmul emits the softmax denominator (replicated) on rows 0-63 and
  the numerator on rows 64-127 -- no row-sum matmul, and the custom-DVE
  reciprocal runs on the base-0 half tile only (it breaks on
  partition-offset APs).
- Both heads of a group share one two-bank [128,1024] sim PSUM tile,
  so ONE exp activation covers them (amortizes ACT per-op overhead);
  Wo rows are host-permuted to match the head->o2T block mapping so
  the pair's two normalize-muls merge into one strided DVE op.
- Software pipelining: the attention phase of pair p is latency-bound,
  so the projection chains of pair p+1 and pair p's output-projection
  chains are interleaved into its group loop. The LAST pair (which has
  no next-pair chains) absorbs the previous pair's trailing y chains
  plus its own held-back q7/k7/v chains, accumulates window-1 y groups
  incrementally as head blocks normalize, and drains its final y store
  through idle sim banks to minimize the tail.

Per-core pipeline (window = 256 tokens, H=16 heads, DH=64):
  qT = Wq.T @ X.T   (lhsT=Wq tiles,  rhs=XT)          [o, i]
  kT = Wk.T @ X.T                                      [o, i]
  v  = X @ Wv       (lhsT=XT tiles,  rhs=Wv)           [j, v|1]
  per (window, head):
    simT = kT_h.T-free @ qT_h   -> [j, i] in PSUM     (j on partitions)
    es   = exp(0.125 * simT)    (ACT, PSUM->SBUF, bf16)
    av   = [v_h|1].T-free @ es  -> [128, i] PSUM
    rS   = 1/S   (one DVE reciprocal per head pair, full PSUM bank)
    o2T  = o2u * rS             (DVE, bf16 [o, i] SBUF)
  Y = o2T.T @ Wo       (lhsT=o2T tiles, rhs=Wo; zero bias added host-side)
"""

import numpy as np
import ml_dtypes

import concourse.bass as bass
import concourse.mybir as mybir
import concourse.tile as tile
from concourse import bacc, bass_utils
from concourse.bass_interp import get_hw_module

H = 16
DH = 64
WIN = 256
D = 1024
B = 4
N = 4096
N_CORES = 8
N_WIN_TOTAL = B * N // WIN          # 64
N_WIN = N_WIN_TOTAL // N_CORES      # 8 windows per core
TOK = N_WIN * WIN                   # 2048 token rows per core
PAIR = 2 * WIN                      # 512 tokens per window pair
SCALE = DH ** -0.5

F32 = mybir.dt.float32
BF16 = mybir.dt.bfloat16


def _body(tc, xqT, wq, wk, wv, wo, out, n_win):
    nc = tc.nc
    from contextlib import ExitStack

    n_pair = n_win // 2

    with ExitStack() as ctx:
        singles = ctx.enter_context(tc.tile_pool(name="singles", bufs=1))
        acts = ctx.enter_context(tc.tile_pool(name="acts", bufs=1))
        heads = ctx.enter_context(tc.tile_pool(name="heads", bufs=3))
        ypool = ctx.enter_context(tc.tile_pool(name="ypool", bufs=2))
        psA = ctx.enter_context(tc.tile_pool(name="psA", bufs=2, space="PSUM"))
        psS = ctx.enter_context(tc.tile_pool(name="psS", bufs=4, space="PSUM"))
        psV = ctx.enter_context(tc.tile_pool(name="psV", bufs=2, space="PSUM"))

        def emit_xt_dma(wp):
            halves = make_xt(wp)
            for hf in range(2):
                dma_xt_half(halves[hf], wp, hf)
            return halves

        # weights in half-tiles; DRAM side is host-prepacked so every DMA is
        # per-partition contiguous (128 descriptors of 8KB) -- the strided
        # rearrange descriptors cost 0.6-5.9us PER dma_start on the sync
        # engine and capped DMA at ~300GB/s in the v1 trace
        wsb = {}
        wdram = {"wq": wq, "wk": wk, "wv": wv, "wo": wo}
        for name in ("wq", "wk", "wv", "wo"):
            wsb[name] = [
                singles.tile([128, 4 * D], BF16, tag=f"{name}{hf}",
                             name=f"sb_{name}_{hf}")
                for hf in range(2)
            ]

        def dma_w_half(name, hf):
            nc.sync.dma_start(
                wsb[name][hf][:],
                wdram[name][:, hf * 4 * D:(hf + 1) * 4 * D],
            )

        def dma_w_chunk(name, hf, klp):
            nc.sync.dma_start(
                wsb[name][hf][:, klp * 2 * D:(klp + 1) * 2 * D],
                wdram[name][:, hf * 4 * D + klp * 2 * D:
                            hf * 4 * D + (klp + 1) * 2 * D],
            )

        def make_xt(wp):
            return [acts.tile([128, 4 * 512], BF16, tag=f"xt{hf}", bufs=2,
                              name=f"xt_{wp}_{hf}") for hf in range(2)]

        def dma_xt_half(t, wp, hf):
            nc.sync.dma_start(
                t[:],
                xqT[:, (wp * 2 + hf) * 2048:(wp * 2 + hf + 1) * 2048],
            )

        def dma_xt_chunk(t, wp, hf, klp):
            nc.sync.dma_start(
                t[:, klp * 1024:(klp + 1) * 1024],
                xqT[:, (wp * 2 + hf) * 2048 + klp * 1024:
                    (wp * 2 + hf) * 2048 + (klp + 1) * 1024],
            )

        # PE warm-up: the HAM clock gate needs ~3.4us of sustained activity
        # to lift the cold 1.2GHz throttle, and real work can't start until
        # DMA delivers data -- so stream dummy matmuls over a zeroed scratch
        # tile while the first chunks arrive, so real chains run warm
        scratch = singles.tile([128, 512], BF16, name="warm_scratch")
        nc.gpsimd.memset(scratch[:], 0.0)
        warm_ps = psV.tile([128, 512], F32, tag="av", name="warm_ps")
        for _ in range(10):
            nc.tensor.matmul(warm_ps[:], scratch[:, 0:128], scratch[:],
                             start=True, stop=True)

        # pair 0's gating transfers stream in chunks, interleaved so each
        # (xt, wq) chunk pair enables the next kl-slices of q matmuls
        xt0 = make_xt(0)
        for klp in range(2):
            dma_xt_chunk(xt0[0], 0, 0, klp)
            dma_w_chunk("wq", 0, klp)
        for klp in range(2):
            dma_xt_chunk(xt0[1], 0, 1, klp)
            dma_w_chunk("wq", 1, klp)
        for name in ("wk", "wv", "wo"):
            for hf in range(2):
                dma_w_half(name, hf)

        # v buffers: pair parity x window -> 4 buffers; per-head layout
        # [v_h (64 cols) | ones (64 cols)] so AV' yields sums on rows 64+.
        v2b = []
        for i in range(4):
            t = singles.tile([128, 2 * H * 128], BF16, name=f"v2_{i}")
            ones_view = t[:].rearrange("p (j h c) -> p j h c", j=2, h=H)[:, :, :, :DH]
            nc.gpsimd.memset(ones_view, 1.0)
            v2b.append(t)

        def proj_chains(wp, xt):
            """qT/kT/v chains for pair wp as a list of zero-arg closures."""
            proj = {}
            for pname in ("qT", "kT"):
                proj[pname] = acts.tile([128, 8 * 512], BF16, tag=pname,
                                        bufs=2, name=f"{pname}_{wp}")
            chains = []
            for ot in range(8):
                for pname, wname in (("qT", "wq"), ("kT", "wk")):
                    def qk_chain(ot=ot, pname=pname, wname=wname):
                        pq = psA.tile([128, 512], F32, tag="acc",
                                      name=f"pq_{wp}_{pname}_{ot}")
                        for kt in range(8):
                            hf, kl = kt // 4, kt % 4
                            nc.tensor.matmul(
                                pq[:],
                                wsb[wname][hf][:, kl * D + ot * 128:
                                               kl * D + (ot + 1) * 128],
                                xt[hf][:, kl * 512:(kl + 1) * 512],
                                start=(kt == 0),
                                stop=(kt == 7),
                            )
                        cp = (nc.vector.tensor_copy if pname == "qT"
                              else nc.scalar.copy)
                        cp(proj[pname][:, ot * 512:(ot + 1) * 512], pq[:])
                    chains.append(qk_chain)
            for tt in range(4):
                for oc in range(2):
                    def v_chain(tt=tt, oc=oc):
                        wl, jt = tt // 2, tt % 2
                        pv = psA.tile([128, 512], F32, tag="acc",
                                      name=f"pv_{wp}_{tt}_{oc}")
                        for kt in range(8):
                            hf, kl = kt // 4, kt % 4
                            nc.tensor.matmul(
                                pv[:],
                                xt[hf][:, kl * 512 + tt * 128:
                                       kl * 512 + (tt + 1) * 128],
                                wsb["wv"][hf][:, kl * D + oc * 512:
                                              kl * D + (oc + 1) * 512],
                                start=(kt == 0),
                                stop=(kt == 7),
                            )
                        vdst = v2b[(wp % 2) * 2 + wl]
                        dsl = vdst[:, jt * H * 128 + oc * 8 * 128:
                                   jt * H * 128 + (oc + 1) * 8 * 128]
                        nc.scalar.copy(
                            dsl.rearrange("p (h c) -> p h c", h=8)[:, :, DH:],
                            pv[:],
                        )
                    chains.append(v_chain)
            return proj, chains

        y_sb_t = {}

        def emit_y_group(wp, o2T, it, ec, pool=None, tag="acc"):
            # both ec halves share one y_sb [128, 1024]; the ec=1 group
            # flushes it with a single contiguous 2KB-per-partition DMA
            row0 = wp * PAIR
            pool = pool or psA
            kw = {"bufs": 4} if tag == "sim" else {}
            py = pool.tile([128, 512], F32, tag=tag,
                           name=f"py_{wp}_{it}_{ec}", **kw)
            for kt2 in range(8):
                hf, kl = kt2 // 4, kt2 % 4
                nc.tensor.matmul(
                    py[:],
                    o2T[:, kt2 * 512 + it * 128:kt2 * 512 + (it + 1) * 128],
                    wsb["wo"][hf][:, kl * D + ec * 512:kl * D + (ec + 1) * 512],
                    start=(kt2 == 0),
                    stop=(kt2 == 7),
                )
            if ec == 0:
                y_sb_t[it] = ypool.tile([128, 1024], BF16, tag="y",
                                        name=f"y_{wp}_{it}")
            y_sb = y_sb_t[it]
            cp = nc.vector.tensor_copy if ec == 0 else nc.scalar.copy
            cp(y_sb[:, ec * 512:(ec + 1) * 512], py[:])
            if ec == 1:
                nc.sync.dma_start(
                    out[row0 + it * 128:row0 + (it + 1) * 128, :],
                    y_sb_t.pop(it)[:],
                )

        deferred_y = []

        def attention(wp, qT, kT, o2T, extra, last=False,
                      defer_trailing=False):
            """16 groups of paired heads (2m, 2m+1) per pair; `extra` chains
            are paced through the loop to keep the PE streaming while DVE/ACT
            normalize. The paired heads' qT/kT rows live on complementary
            partition halves, so their sim matmuls issue adjacently as
            64x128 row-tiles T(0,0)/T(64,0) and stream CONCURRENTLY --
            halving sim cost vs serial emission. Head h=2m+s maps to o2T
            block s*4+m//2, rows 64*(m%2) (Wo rows are host-permuted to
            match), so the pair's two normalize-muls still merge into one
            stride-4 two-block DVE op."""
            G = 16
            es_t = {}
            av_t = {}

            def emit_sim(g):
                wl, m = g // 8, g % 8
                ocol = m * 512 + wl * WIN
                # one two-bank PSUM tile per group: both heads' sims land in
                # [128, 1024] and ONE exp activation covers them (the ~260ns
                # per-op ACT overhead amortizes over 2 heads)
                ps = psS.tile([128, 1024], F32, tag="sim", bufs=2,
                              name=f"sim_{wp}_{g}")
                for jt in range(2):
                    for s in range(2):
                        prow = s * 64
                        nc.tensor.matmul(
                            ps[:, s * 512 + jt * WIN:s * 512 + (jt + 1) * WIN],
                            kT[prow:prow + 64,
                               ocol + jt * 128:ocol + (jt + 1) * 128],
                            qT[prow:prow + 64, ocol:ocol + WIN],
                            start=True,
                            stop=True,
                        )
                e = heads.tile([128, 1024], BF16, tag="es", bufs=4,
                               name=f"es_{wp}_{g}")
                nc.scalar.activation(
                    e[:], ps[:], mybir.ActivationFunctionType.Exp,
                    scale=SCALE,
                )
                es_t[g] = e

            def emit_av(g):
                wl, m = g // 8, g % 8
                av2 = psV.tile([128, 512], F32, tag="av", name=f"av_{wp}_{g}")
                es = es_t.pop(g)
                for s in range(2):
                    h = 2 * m + s
                    for jt in range(2):
                        nc.tensor.matmul(
                            av2[:, s * WIN:(s + 1) * WIN],
                            v2b[(wp % 2) * 2 + wl][:, (jt * H + h) * 128:
                                                   (jt * H + h + 1) * 128],
                            es[:, s * 512 + jt * WIN:s * 512 + (jt + 1) * WIN],
                            start=(jt == 0),
                            stop=(jt == 1),
                        )
                av_t[g] = av2

            def emit_epilogue(g):
                wl, m = g // 8, g % 8
                av2 = av_t.pop(g)
                rs = heads.tile([128, 512], F32, tag="rs", name=f"rs_{wp}_{g}")
                # sums live on rows 0:64 (ones-first v2b layout) so the
                # custom-DVE reciprocal runs on a base-0 half tile
                nc.vector.reciprocal_approx_fast(rs[0:64, :], av2[0:64, :])
                r0 = 64 * (m % 2)
                dst = o2T[r0:r0 + 64, :] \
                    .rearrange("p (b c) -> p b c", c=512) \
                    [:, (m // 2)::4, wl * WIN:(wl + 1) * WIN]
                nc.vector.tensor_mul(
                    dst,
                    av2[64:128, :].rearrange("p (b c) -> p b c", b=2),
                    rs[0:64, :].rearrange("p (b c) -> p b c", b=2),
                )

            # last pair only: window-1 y groups (it=2) accumulate
            # incrementally as head blocks normalize, trimming the tail
            y_inc = {}

            def emit_y_inc(g):
                row0 = wp * PAIR
                b = (g - 9) // 2
                for ec in range(2):
                    if g == 9:
                        y_inc[ec] = psA.tile([128, 512], F32, tag="acc",
                                             name=f"pyi_{wp}_{ec}")
                    for kt2 in (b, b + 4):
                        hf, kl = kt2 // 4, kt2 % 4
                        nc.tensor.matmul(
                            y_inc[ec][:],
                            o2T[:, kt2 * 512 + 2 * 128:kt2 * 512 + 3 * 128],
                            wsb["wo"][hf][:, kl * D + ec * 512:
                                          kl * D + (ec + 1) * 512],
                            start=(g == 9 and kt2 == b),
                            stop=(g == 15 and kt2 == b + 4),
                        )

            def flush_y_inc():
                row0 = wp * PAIR
                y_sb = ypool.tile([128, 1024], BF16, tag="y",
                                  name=f"y_{wp}_2")
                for ec in range(2):
                    cp = nc.vector.tensor_copy if ec == 0 else nc.scalar.copy
                    cp(y_sb[:, ec * 512:(ec + 1) * 512], y_inc[ec][:])
                nc.sync.dma_start(out[row0 + 256:row0 + 384, :], y_sb[:])

            n_extra = len(extra)
            ch_i = 0
            # two groups of sims (then avs) per batch: halves the number of
            # 64x128 <-> 128x128 tiling-mode transitions on the PE array
            emit_sim(0)
            emit_sim(1)
            for g in range(0, G, 2):
                if g == 8:
                    # must precede y_inc(9): it reuses the same psA banks,
                    # and y_inc holds them with open groups through g15
                    emit_y_group(wp, o2T, 1, 0)
                    emit_y_group(wp, o2T, 1, 1)
                for gg in (g, g + 1):
                    emit_av(gg)
                    emit_epilogue(gg)
                    if last and gg in (9, 11, 13, 15):
                        emit_y_inc(gg)
                if g + 2 < G:
                    emit_sim(g + 2)
                    emit_sim(g + 3)
                if g == 6:
                    # window 0 fully normalized at group 7: flow its Y groups
                    emit_y_group(wp, o2T, 0, 0)
                    emit_y_group(wp, o2T, 0, 1)
                # last pair: its extras are psA-bank y chains that must all
                # retire before y_inc claims those banks at group 9
                pace = 10 if last else G
                while ch_i < n_extra and ch_i * pace < n_extra * (g + 2):
                    extra[ch_i]()
                    ch_i += 1
            if last:
                # both it3 chains run in an idle wide sim bank so the tail
                # never waits on psA (held by y_inc until its flush)
                row0 = wp * PAIR
                pw = psS.tile([128, 1024], F32, tag="sim", bufs=2,
                              name=f"py3_{wp}")
                for ec in range(2):
                    for kt2 in range(8):
                        hf, kl = kt2 // 4, kt2 % 4
                        nc.tensor.matmul(
                            pw[:, ec * 512:(ec + 1) * 512],
                            o2T[:, kt2 * 512 + 3 * 128:kt2 * 512 + 4 * 128],
                            wsb["wo"][hf][:, kl * D + ec * 512:
                                          kl * D + (ec + 1) * 512],
                            start=(kt2 == 0),
                            stop=(kt2 == 7),
                        )
                flush_y_inc()
                # split the final store so the ec0 half flies while the ec1
                # chain/copy finishes -- shrinks the post-last-matmul tail
                y_sb = ypool.tile([128, 1024], BF16, tag="y", name=f"y_{wp}_3")
                nc.vector.tensor_copy(y_sb[:, 0:512], pw[:, 0:512])
                nc.sync.dma_start(out[row0 + 384:row0 + 512, 0:512],
                                  y_sb[:, 0:512])
                nc.scalar.copy(y_sb[:, 512:1024], pw[:, 512:1024])
                nc.sync.dma_start(out[row0 + 384:row0 + 512, 512:1024],
                                  y_sb[:, 512:1024])
            elif defer_trailing:
                # hand this pair's trailing y chains to the LAST pair's
                # attention loop -- it has no projection chains to pace, so
                # these fill its dependency-latency gaps
                for it in range(2, 4):
                    for ec in range(2):
                        deferred_y.append(
                            lambda it=it, ec=ec, o2T=o2T, wp=wp:
                            emit_y_group(wp, o2T, it, ec))
            else:
                for it in range(2, 4):
                    emit_y_group(wp, o2T, it, 0)
                    emit_y_group(wp, o2T, it, 1)

        proj, chains0 = proj_chains(0, xt0)
        # pair 0 q chains: kl-major sweep holding all 8 PSUM banks (nothing
        # else uses PSUM this early) so the PE consumes each 256KB chunk of
        # wq/xt the moment its DMA lands
        pq0 = []
        pq0_wide = [psS.tile([128, 1024], F32, tag="sim", bufs=2,
                             name=f"pq0_w_{i}") for i in range(2)]
        for ot in range(8):
            if ot < 2:
                pq0.append(psA.tile([128, 512], F32, tag="acc",
                                    name=f"pq0_qT_{ot}"))
            elif ot < 6:
                w = pq0_wide[(ot - 2) // 2]
                pq0.append(w[:, ((ot - 2) % 2) * 512:((ot - 2) % 2 + 1) * 512])
            else:
                pq0.append(psV.tile([128, 512], F32, tag="av",
                                    name=f"pq0_qT_{ot}"))
        for hf in range(2):
            for kl in range(4):
                kt = hf * 4 + kl
                for ot in range(8):
                    nc.tensor.matmul(
                        pq0[ot][:],
                        wsb["wq"][hf][:, kl * D + ot * 128:
                                      kl * D + (ot + 1) * 128],
                        xt0[hf][:, kl * 512:(kl + 1) * 512],
                        start=(kt == 0),
                        stop=(kt == 7),
                    )
        for ot in range(8):
            # alternate engines so the eight copies don't serialize on DVE
            cp = nc.vector.tensor_copy if ot % 2 == 0 else nc.scalar.copy
            cp(proj["qT"][:, ot * 512:(ot + 1) * 512], pq0[ot][:])
        for ch in chains0[1::2][:8] + chains0[16:]:
            ch()
        held = []
        for wp in range(n_pair):
            o2T = acts.tile([128, 8 * 512], BF16, tag="o2T", bufs=2,
                            name=f"o2T_{wp}")
            if wp + 1 < n_pair:
                xt_next = emit_xt_dma(wp + 1)
                proj_next, chains_next = proj_chains(wp + 1, xt_next)
                if wp + 2 == n_pair and n_pair > 1:
                    # hold back the last pair's latest-needed chains (q7/k7
                    # feed sims for heads 14/15; the oc1 window-1 v chains
                    # feed avs from group 12) to fill its latency gaps
                    held = [chains_next[i] for i in (14, 15, 21, 23)]
                    chains_next = [c for i, c in enumerate(chains_next)
                                   if i not in (14, 15, 21, 23)]
            else:
                proj_next, chains_next = None, held + deferred_y
            attention(wp, proj["qT"], proj["kT"], o2T, chains_next,
                      last=(wp + 1 == n_pair),
                      defer_trailing=(wp + 2 == n_pair))
            proj = proj_next


_CACHE = {}


def _build(n_win=N_WIN):
    key = n_win
    if key in _CACHE:
        return _CACHE[key]
    tok = n_win * WIN
    nc = bacc.Bacc(
        "TRN2", target_bir_lowering=False, debug=False, num_devices=N_CORES
    )
    # host-prepacked layouts: every DMA source is per-partition contiguous
    xqT = nc.dram_tensor("xqT", [128, tok * 8], BF16, kind="ExternalInput").ap()
    wq = nc.dram_tensor("Wq", [128, 8 * D], BF16, kind="ExternalInput").ap()
    wk = nc.dram_tensor("Wk", [128, 8 * D], BF16, kind="ExternalInput").ap()
    wv = nc.dram_tensor("Wv", [128, 8 * D], BF16, kind="ExternalInput").ap()
    wo = nc.dram_tensor("Wo", [128, 8 * D], BF16, kind="ExternalInput").ap()
    out = nc.dram_tensor("out", [tok, D], BF16, kind="ExternalOutput").ap()
    with tile.TileContext(nc) as tc:
        _body(tc, xqT, wq, wk, wv, wo, out, n_win)
    nc.compile()
    nc.m = get_hw_module(nc.m)
    _CACHE[key] = nc
    return nc


def _pack_w(W):
    # wpre[p, hf*4096 + kt*1024 + c] = W[hf*512 + kt*128 + p, c]
    bf = ml_dtypes.bfloat16
    W = np.asarray(W, np.float32).astype(bf)
    return np.ascontiguousarray(
        W.reshape(2, 4, 128, D).transpose(2, 0, 1, 3).reshape(128, 8 * D)
    )


def _wo_perm():
    # o2T index of head h=2m+s, dim d is (s*4 + m//2)*128 + 64*(m%2) + d;
    # permute Wo rows so the y matmul contracts o2T directly
    perm = np.empty(D, np.int64)
    for h in range(H):
        m, s = h // 2, h % 2
        base = (s * 4 + m // 2) * 128 + 64 * (m % 2)
        perm[base:base + DH] = np.arange(h * DH, (h + 1) * DH)
    return perm


def _pack_x(q2, n_pair):
    # xpre[p, (wp*2+hf)*2048 + dt*512 + t] = X[wp*512 + t, hf*512 + dt*128 + p]
    return np.ascontiguousarray(
        q2.reshape(n_pair, 512, 2, 4, 128)
        .transpose(4, 0, 2, 3, 1)
        .reshape(128, n_pair * 4096)
    )


def run(query, Wq, Wk, Wv, Wo, bo, n_win=N_WIN, **spmd_kwargs):
    nc = _build(n_win)
    tok = n_win * WIN
    n_pair = n_win // 2
    bf = ml_dtypes.bfloat16
    q2 = np.asarray(query, dtype=np.float32).reshape(-1, D).astype(bf)
    weights = {
        "Wq": _pack_w(Wq),
        "Wk": _pack_w(Wk),
        "Wv": _pack_w(Wv),
        "Wo": _pack_w(np.asarray(Wo, np.float32)[_wo_perm()]),
    }
    in_maps = []
    for c in range(N_CORES):
        m = {"xqT": _pack_x(q2[c * TOK:c * TOK + tok], n_pair)}
        m.update(weights)
        in_maps.append(m)
    try:
        res = bass_utils.run_bass_kernel_spmd(
            nc, in_maps, core_ids=list(range(N_CORES)), **spmd_kwargs
        )
    except Exception:
        # transient NRT_EXEC_UNIT_UNRECOVERABLE wedges clear on retry
        res = bass_utils.run_bass_kernel_spmd(
            nc, in_maps, core_ids=list(range(N_CORES)), **spmd_kwargs
        )
    outs = [np.asarray(res.results[c]["out"]).astype(np.float32)
            for c in range(N_CORES)]
    return outs, res


def kernel(query, context, Wq, Wk, Wv, Wo, bo):
    outs, _ = run(query, Wq, Wk, Wv, Wo, bo)
    y = np.concatenate(outs, axis=0).reshape(B, N, D)
    bo = np.asarray(bo, np.float32)
    if bo.any():
        y = y + bo  # bias is structurally zero for this problem; host-add keeps exactness
    return y.astype(np.float32)



# revision 43
# speedup vs baseline: 1.0083x; 1.0002x over previous
"""Windowed local self-attention (CrossAttention module with the context-
overwrite bug faithfully reproduced) on 8 Trainium2 NeuronCores.

Full-input contract: kernel(**inputs) takes the unsharded tensors and
returns the full (4, 4096, 1024) output. Internally the 64 independent
windows of 256 tokens are data-parallel sharded 8-per-core; the four
projection weights are broadcast to every core. No collectives needed.

All matmul operands are bf16 (host-cast): 1 cycle/row on the PE, half
the SBUF/DMA traffic of fp32, far less PE power than fp32 HIGH mode
(which triggered 50% periodic throttling in the fp32r version). PSUM
accumulation, softmax normalization and the final output stay fp32.

Key structure:
- X and all weights are repacked on the HOST so every DMA source is
  per-partition contiguous (128 x 8KB descriptors; ~430 GB/s vs ~300
  for strided patterns) -- no PE transposes / identity preamble at all.
- Warm-up: dummy matmuls over a zeroed scratch tile lift the HAM clock
  throttle (1.2 -> 2.4 GHz needs ~3.4us sustained PE activity) while
  the first DMA chunks stream in; pair-0 q chains are kl-chunk-gated.
- Windows processed in PAIRS (512 tokens): every projection/output
  matmul streams the max 512 moving rows, hiding LDWEIGHTS.
- Heads paired (2m, 2m+1) live on complementary qT/kT partition halves:
  their sim matmuls issue adjacently as 64x128 row-tiles T(0,0)/T(64,0)
  and stream CONCURRENTLY (2x sim throughput). Two groups of sims/avs
  are batched per stretch to halve tiling-mode-switch bubbles.
- V is stored interleaved per head as [ones (64) | v_h (64)]; the AV
  matmul emits the softmax denominator (replicated) on rows 0-63 and
  the numerator on rows 64-127 -- no row-sum matmul, and the custom-DVE
  reciprocal runs on the base-0 half tile only (it breaks on
  partition-offset APs).
- Both heads of a group share one two-bank [128,1024] sim PSUM tile,
  so ONE exp activation covers them (amortizes ACT per-op overhead);
  Wo rows are host-permuted to match the head->o2T block mapping so
  the pair's two normalize-muls merge into one strided DVE op.
- Software pipelining: the attention phase of pair p is latency-bound,
  so the projection chains of pair p+1 and pair p's output-projection
  chains are interleaved into its group loop. The LAST pair (which has
  no next-pair chains) absorbs the previous pair's trailing y chains
  plus its own held-back q7/k7/v chains, accumulates window-1 y groups
  incrementally as head blocks normalize, and drains its final y store
  through idle sim banks to minimize the tail.

Per-core pipeline (window = 256 tokens, H=16 heads, DH=64):
  qT = Wq.T @ X.T   (lhsT=Wq tiles,  rhs=XT)          [o, i]
  kT = Wk.T @ X.T                                      [o, i]
  v  = X @ Wv       (lhsT=XT tiles,  rhs=Wv)           [j, v|1]
  per (window, head):
    simT = kT_h.T-free @ qT_h   -> [j, i] in PSUM     (j on partitions)
    es   = exp(0.125 * simT)    (ACT, PSUM->SBUF, bf16)
    av   = [v_h|1].T-free @ es  -> [128, i] PSUM
    rS   = 1/S   (one DVE reciprocal per head pair, full PSUM bank)
    o2T  = o2u * rS             (DVE, bf16 [o, i] SBUF)
  Y = o2T.T @ Wo       (lhsT=o2T tiles, rhs=Wo; zero bias added host-side)
"""

import numpy as np
import ml_dtypes

import concourse.bass as bass
import concourse.mybir as mybir
import concourse.tile as tile
from concourse import bacc, bass_utils
from concourse.bass_interp import get_hw_module

H = 16
DH = 64
WIN = 256
D = 1024
B = 4
N = 4096
N_CORES = 8
N_WIN_TOTAL = B * N // WIN          # 64
N_WIN = N_WIN_TOTAL // N_CORES      # 8 windows per core
TOK = N_WIN * WIN                   # 2048 token rows per core
PAIR = 2 * WIN                      # 512 tokens per window pair
SCALE = DH ** -0.5

F32 = mybir.dt.float32
BF16 = mybir.dt.bfloat16


def _body(tc, xqT, wq, wk, wv, wo, out, n_win):
    nc = tc.nc
    from contextlib import ExitStack

    n_pair = n_win // 2

    with ExitStack() as ctx:
        singles = ctx.enter_context(tc.tile_pool(name="singles", bufs=1))
        acts = ctx.enter_context(tc.tile_pool(name="acts", bufs=1))
        heads = ctx.enter_context(tc.tile_pool(name="heads", bufs=3))
        ypool = ctx.enter_context(tc.tile_pool(name="ypool", bufs=2))
        psA = ctx.enter_context(tc.tile_pool(name="psA", bufs=2, space="PSUM"))
        psS = ctx.enter_context(tc.tile_pool(name="psS", bufs=4, space="PSUM"))
        psV = ctx.enter_context(tc.tile_pool(name="psV", bufs=2, space="PSUM"))

        def emit_xt_dma(wp):
            halves = make_xt(wp)
            for hf in range(2):
                dma_xt_half(halves[hf], wp, hf)
            return halves

        # weights in half-tiles; DRAM side is host-prepacked so every DMA is
        # per-partition contiguous (128 descriptors of 8KB) -- the strided
        # rearrange descriptors cost 0.6-5.9us PER dma_start on the sync
        # engine and capped DMA at ~300GB/s in the v1 trace
        wsb = {}
        wdram = {"wq": wq, "wk": wk, "wv": wv, "wo": wo}
        for name in ("wq", "wk", "wv", "wo"):
            wsb[name] = [
                singles.tile([128, 4 * D], BF16, tag=f"{name}{hf}",
                             name=f"sb_{name}_{hf}")
                for hf in range(2)
            ]

        def dma_w_half(name, hf):
            nc.sync.dma_start(
                wsb[name][hf][:],
                wdram[name][:, hf * 4 * D:(hf + 1) * 4 * D],
            )

        def dma_w_chunk(name, hf, klp):
            nc.sync.dma_start(
                wsb[name][hf][:, klp * 2 * D:(klp + 1) * 2 * D],
                wdram[name][:, hf * 4 * D + klp * 2 * D:
                            hf * 4 * D + (klp + 1) * 2 * D],
            )

        def make_xt(wp):
            return [acts.tile([128, 4 * 512], BF16, tag=f"xt{hf}", bufs=2,
                              name=f"xt_{wp}_{hf}") for hf in range(2)]

        def dma_xt_half(t, wp, hf):
            nc.sync.dma_start(
                t[:],
                xqT[:, (wp * 2 + hf) * 2048:(wp * 2 + hf + 1) * 2048],
            )

        def dma_xt_chunk(t, wp, hf, klp):
            nc.sync.dma_start(
                t[:, klp * 1024:(klp + 1) * 1024],
                xqT[:, (wp * 2 + hf) * 2048 + klp * 1024:
                    (wp * 2 + hf) * 2048 + (klp + 1) * 1024],
            )

        # PE warm-up: the HAM clock gate needs ~3.4us of sustained activity
        # to lift the cold 1.2GHz throttle, and real work can't start until
        # DMA delivers data -- so stream dummy matmuls over a zeroed scratch
        # tile while the first chunks arrive, so real chains run warm
        scratch = singles.tile([128, 512], BF16, name="warm_scratch")
        nc.gpsimd.memset(scratch[:], 0.0)
        warm_ps = psV.tile([128, 512], F32, tag="av", name="warm_ps")
        for _ in range(10):
            nc.tensor.matmul(warm_ps[:], scratch[:, 0:128], scratch[:],
                             start=True, stop=True)

        # pair 0's gating transfers stream in chunks, interleaved so each
        # (xt, wq) chunk pair enables the next kl-slices of q matmuls
        xt0 = make_xt(0)
        for klp in range(2):
            dma_xt_chunk(xt0[0], 0, 0, klp)
            dma_w_chunk("wq", 0, klp)
        for klp in range(2):
            dma_xt_chunk(xt0[1], 0, 1, klp)
            dma_w_chunk("wq", 1, klp)
        for name in ("wk", "wv", "wo"):
            for hf in range(2):
                dma_w_half(name, hf)

        # v buffers: pair parity x window -> 4 buffers; per-head layout
        # [v_h (64 cols) | ones (64 cols)] so AV' yields sums on rows 64+.
        v2b = []
        for i in range(4):
            t = singles.tile([128, 2 * H * 128], BF16, name=f"v2_{i}")
            ones_view = t[:].rearrange("p (j h c) -> p j h c", j=2, h=H)[:, :, :, :DH]
            nc.gpsimd.memset(ones_view, 1.0)
            v2b.append(t)

        def proj_chains(wp, xt):
            """qT/kT/v chains for pair wp as a list of zero-arg closures."""
            proj = {}
            for pname in ("qT", "kT"):
                proj[pname] = acts.tile([128, 8 * 512], BF16, tag=pname,
                                        bufs=2, name=f"{pname}_{wp}")
            chains = []
            for ot in range(8):
                for pname, wname in (("qT", "wq"), ("kT", "wk")):
                    def qk_chain(ot=ot, pname=pname, wname=wname):
                        pq = psA.tile([128, 512], F32, tag="acc",
                                      name=f"pq_{wp}_{pname}_{ot}")
                        for kt in range(8):
                            hf, kl = kt // 4, kt % 4
                            nc.tensor.matmul(
                                pq[:],
                                wsb[wname][hf][:, kl * D + ot * 128:
                                               kl * D + (ot + 1) * 128],
                                xt[hf][:, kl * 512:(kl + 1) * 512],
                                start=(kt == 0),
                                stop=(kt == 7),
                            )
                        cp = (nc.vector.tensor_copy if pname == "qT"
                              else nc.scalar.copy)
                        cp(proj[pname][:, ot * 512:(ot + 1) * 512], pq[:])
                    chains.append(qk_chain)
            for tt in range(4):
                for oc in range(2):
                    def v_chain(tt=tt, oc=oc):
                        wl, jt = tt // 2, tt % 2
                        pv = psA.tile([128, 512], F32, tag="acc",
                                      name=f"pv_{wp}_{tt}_{oc}")
                        for kt in range(8):
                            hf, kl = kt // 4, kt % 4
                            nc.tensor.matmul(
                                pv[:],
                                xt[hf][:, kl * 512 + tt * 128:
                                       kl * 512 + (tt + 1) * 128],
                                wsb["wv"][hf][:, kl * D + oc * 512:
                                              kl * D + (oc + 1) * 512],
                                start=(kt == 0),
                                stop=(kt == 7),
                            )
                        vdst = v2b[(wp % 2) * 2 + wl]
                        dsl = vdst[:, jt * H * 128 + oc * 8 * 128:
                                   jt * H * 128 + (oc + 1) * 8 * 128]
                        nc.scalar.copy(
                            dsl.rearrange("p (h c) -> p h c", h=8)[:, :, DH:],
                            pv[:],
                        )
                    chains.append(v_chain)
            return proj, chains

        y_sb_t = {}

        def emit_y_group(wp, o2T, it, ec, pool=None, tag="acc"):
            # both ec halves share one y_sb [128, 1024]; the ec=1 group
            # flushes it with a single contiguous 2KB-per-partition DMA
            row0 = wp * PAIR
            pool = pool or psA
            kw = {"bufs": 4} if tag == "sim" else {}
            py = pool.tile([128, 512], F32, tag=tag,
                           name=f"py_{wp}_{it}_{ec}", **kw)
            for kt2 in range(8):
                hf, kl = kt2 // 4, kt2 % 4
                nc.tensor.matmul(
                    py[:],
                    o2T[:, kt2 * 512 + it * 128:kt2 * 512 + (it + 1) * 128],
                    wsb["wo"][hf][:, kl * D + ec * 512:kl * D + (ec + 1) * 512],
                    start=(kt2 == 0),
                    stop=(kt2 == 7),
                )
            if ec == 0:
                y_sb_t[it] = ypool.tile([128, 1024], BF16, tag="y",
                                        name=f"y_{wp}_{it}")
            y_sb = y_sb_t[it]
            cp = nc.vector.tensor_copy if ec == 0 else nc.scalar.copy
            cp(y_sb[:, ec * 512:(ec + 1) * 512], py[:])
            if ec == 1:
                nc.sync.dma_start(
                    out[row0 + it * 128:row0 + (it + 1) * 128, :],
                    y_sb_t.pop(it)[:],
                )

        deferred_y = []

        def attention(wp, qT, kT, o2T, extra, last=False,
                      defer_trailing=False):
            """16 groups of paired heads (2m, 2m+1) per pair; `extra` chains
            are paced through the loop to keep the PE streaming while DVE/ACT
            normalize. The paired heads' qT/kT rows live on complementary
            partition halves, so their sim matmuls issue adjacently as
            64x128 row-tiles T(0,0)/T(64,0) and stream CONCURRENTLY --
            halving sim cost vs serial emission. Head h=2m+s maps to o2T
            block s*4+m//2, rows 64*(m%2) (Wo rows are host-permuted to
            match), so the pair's two normalize-muls still merge into one
            stride-4 two-block DVE op."""
            G = 16
            es_t = {}
            av_t = {}

            def emit_sim(g):
                wl, m = g // 8, g % 8
                ocol = m * 512 + wl * WIN
                # one two-bank PSUM tile per group: both heads' sims land in
                # [128, 1024] and ONE exp activation covers them (the ~260ns
                # per-op ACT overhead amortizes over 2 heads)
                ps = psS.tile([128, 1024], F32, tag="sim", bufs=2,
                              name=f"sim_{wp}_{g}")
                for jt in range(2):
                    for s in range(2):
                        prow = s * 64
                        nc.tensor.matmul(
                            ps[:, s * 512 + jt * WIN:s * 512 + (jt + 1) * WIN],
                            kT[prow:prow + 64,
                               ocol + jt * 128:ocol + (jt + 1) * 128],
                            qT[prow:prow + 64, ocol:ocol + WIN],
                            start=True,
                            stop=True,
                        )
                e = heads.tile([128, 1024], BF16, tag="es", bufs=4,
                               name=f"es_{wp}_{g}")
                nc.scalar.activation(
                    e[:], ps[:], mybir.ActivationFunctionType.Exp,
                    scale=SCALE,
                )
                es_t[g] = e

            def emit_av(g):
                wl, m = g // 8, g % 8
                av2 = psV.tile([128, 512], F32, tag="av", name=f"av_{wp}_{g}")
                es = es_t.pop(g)
                for s in range(2):
                    h = 2 * m + s
                    for jt in range(2):
                        nc.tensor.matmul(
                            av2[:, s * WIN:(s + 1) * WIN],
                            v2b[(wp % 2) * 2 + wl][:, (jt * H + h) * 128:
                                                   (jt * H + h + 1) * 128],
                            es[:, s * 512 + jt * WIN:s * 512 + (jt + 1) * WIN],
                            start=(jt == 0),
                            stop=(jt == 1),
                        )
                av_t[g] = av2

            def emit_epilogue(g):
                wl, m = g // 8, g % 8
                av2 = av_t.pop(g)
                rs = heads.tile([128, 512], F32, tag="rs", name=f"rs_{wp}_{g}")
                # sums live on rows 0:64 (ones-first v2b layout) so the
                # custom-DVE reciprocal runs on a base-0 half tile
                nc.vector.reciprocal_approx_fast(rs[0:64, :], av2[0:64, :])
                r0 = 64 * (m % 2)
                dst = o2T[r0:r0 + 64, :] \
                    .rearrange("p (b c) -> p b c", c=512) \
                    [:, (m // 2)::4, wl * WIN:(wl + 1) * WIN]
                nc.vector.tensor_mul(
                    dst,
                    av2[64:128, :].rearrange("p (b c) -> p b c", b=2),
                    rs[0:64, :].rearrange("p (b c) -> p b c", b=2),
                )

            # last pair only: window-1 y groups (it=2) accumulate
            # incrementally as head blocks normalize, trimming the tail
            y_inc = {}

            Y_INC_KT2 = {11: (0, 4, 1, 5), 13: (2, 6), 15: (3, 7)}

            def emit_y_inc(g):
                # starts at g11 (blocks 0,4,1,5 all normalized) so the psA
                # banks stay free through g10 for the late-paced extras
                row0 = wp * PAIR
                for ec in range(2):
                    if g == 11:
                        y_inc[ec] = psA.tile([128, 512], F32, tag="acc",
                                             name=f"pyi_{wp}_{ec}")
                    for kt2 in Y_INC_KT2[g]:
                        hf, kl = kt2 // 4, kt2 % 4
                        nc.tensor.matmul(
                            y_inc[ec][:],
                            o2T[:, kt2 * 512 + 2 * 128:kt2 * 512 + 3 * 128],
                            wsb["wo"][hf][:, kl * D + ec * 512:
                                          kl * D + (ec + 1) * 512],
                            start=(g == 11 and kt2 == 0),
                            stop=(g == 15 and kt2 == 7),
                        )

            def flush_y_inc():
                row0 = wp * PAIR
                y_sb = ypool.tile([128, 1024], BF16, tag="y",
                                  name=f"y_{wp}_2")
                for ec in range(2):
                    cp = nc.vector.tensor_copy if ec == 0 else nc.scalar.copy
                    cp(y_sb[:, ec * 512:(ec + 1) * 512], y_inc[ec][:])
                nc.sync.dma_start(out[row0 + 256:row0 + 384, :], y_sb[:])

            n_extra = len(extra)
            ch_i = 0
            # two groups of sims (then avs) per batch: halves the number of
            # 64x128 <-> 128x128 tiling-mode transitions on the PE array
            emit_sim(0)
            emit_sim(1)
            # last pair: explicit schedule -- extras are psA-bank chains;
            # the v chains (only read from group 12) run late at g8/g10 to
            # fill the latency gaps y_inc's bank-hold used to force idle
            EX_SCHED = (0, 0, 2, 2, 4, 4, 8, 10)
            for g in range(0, G, 2):
                if last:
                    for i in range(n_extra):
                        if i < len(EX_SCHED) and EX_SCHED[i] == g:
                            extra[i]()
                if g == 8:
                    # must precede y_inc: it reuses the same psA banks and
                    # holds them with open groups through g15
                    emit_y_group(wp, o2T, 1, 0)
                    emit_y_group(wp, o2T, 1, 1)
                for gg in (g, g + 1):
                    emit_av(gg)
                    emit_epilogue(gg)
                    if last and gg in (11, 13, 15):
                        emit_y_inc(gg)
                if g + 2 < G:
                    emit_sim(g + 2)
                    emit_sim(g + 3)
                if g == 6:
                    # window 0 fully normalized at group 7: flow its Y groups
                    emit_y_group(wp, o2T, 0, 0)
                    emit_y_group(wp, o2T, 0, 1)
                if not last:
                    while ch_i < n_extra and ch_i * G < n_extra * (g + 2):
                        extra[ch_i]()
                        ch_i += 1
            if last:
                # both it3 chains run in an idle wide sim bank so the tail
                # never waits on psA (held by y_inc until its flush)
                row0 = wp * PAIR
                pw = psS.tile([128, 1024], F32, tag="sim", bufs=2,
                              name=f"py3_{wp}")
                for ec in range(2):
                    for kt2 in range(8):
                        hf, kl = kt2 // 4, kt2 % 4
                        nc.tensor.matmul(
                            pw[:, ec * 512:(ec + 1) * 512],
                            o2T[:, kt2 * 512 + 3 * 128:kt2 * 512 + 4 * 128],
                            wsb["wo"][hf][:, kl * D + ec * 512:
                                          kl * D + (ec + 1) * 512],
                            start=(kt2 == 0),
                            stop=(kt2 == 7),
                        )
                flush_y_inc()
                # split the final store so the ec0 half flies while the ec1
                # chain/copy finishes -- shrinks the post-last-matmul tail
                y_sb = ypool.tile([128, 1024], BF16, tag="y", name=f"y_{wp}_3")
                nc.vector.tensor_copy(y_sb[:, 0:512], pw[:, 0:512])
                nc.sync.dma_start(out[row0 + 384:row0 + 512, 0:512],
                                  y_sb[:, 0:512])
                nc.scalar.copy(y_sb[:, 512:1024], pw[:, 512:1024])
                nc.sync.dma_start(out[row0 + 384:row0 + 512, 512:1024],
                                  y_sb[:, 512:1024])
            elif defer_trailing:
                # hand this pair's trailing y chains to the LAST pair's
                # attention loop -- it has no projection chains to pace, so
                # these fill its dependency-latency gaps
                for it in range(2, 4):
                    for ec in range(2):
                        deferred_y.append(
                            lambda it=it, ec=ec, o2T=o2T, wp=wp:
                            emit_y_group(wp, o2T, it, ec))
            else:
                for it in range(2, 4):
                    emit_y_group(wp, o2T, it, 0)
                    emit_y_group(wp, o2T, it, 1)

        proj, chains0 = proj_chains(0, xt0)
        # pair 0 q chains: kl-major sweep holding all 8 PSUM banks (nothing
        # else uses PSUM this early) so the PE consumes each 256KB chunk of
        # wq/xt the moment its DMA lands
        pq0 = []
        pq0_wide = [psS.tile([128, 1024], F32, tag="sim", bufs=2,
                             name=f"pq0_w_{i}") for i in range(2)]
        for ot in range(8):
            if ot < 2:
                pq0.append(psA.tile([128, 512], F32, tag="acc",
                                    name=f"pq0_qT_{ot}"))
            elif ot < 6:
                w = pq0_wide[(ot - 2) // 2]
                pq0.append(w[:, ((ot - 2) % 2) * 512:((ot - 2) % 2 + 1) * 512])
            else:
                pq0.append(psV.tile([128, 512], F32, tag="av",
                                    name=f"pq0_qT_{ot}"))
        for hf in range(2):
            for kl in range(4):
                kt = hf * 4 + kl
                for ot in range(8):
                    nc.tensor.matmul(
                        pq0[ot][:],
                        wsb["wq"][hf][:, kl * D + ot * 128:
                                      kl * D + (ot + 1) * 128],
                        xt0[hf][:, kl * 512:(kl + 1) * 512],
                        start=(kt == 0),
                        stop=(kt == 7),
                    )
        for ot in range(8):
            # alternate engines so the eight copies don't serialize on DVE
            cp = nc.vector.tensor_copy if ot % 2 == 0 else nc.scalar.copy
            cp(proj["qT"][:, ot * 512:(ot + 1) * 512], pq0[ot][:])
        for ch in chains0[1::2][:8] + chains0[16:]:
            ch()
        held, held_v = [], []
        for wp in range(n_pair):
            o2T = acts.tile([128, 8 * 512], BF16, tag="o2T", bufs=2,
                            name=f"o2T_{wp}")
            if wp + 1 < n_pair:
                xt_next = emit_xt_dma(wp + 1)
                proj_next, chains_next = proj_chains(wp + 1, xt_next)
                if wp + 2 == n_pair and n_pair > 1:
                    # hold back the last pair's latest-needed chains (q7/k7
                    # feed sims for heads 14/15; the oc1 window-1 v chains
                    # feed avs from group 12) to fill its latency gaps
                    held = [chains_next[i] for i in (14, 15)]
                    held_v = [chains_next[i] for i in (21, 23)]
                    chains_next = [c for i, c in enumerate(chains_next)
                                   if i not in (14, 15, 21, 23)]
            else:
                # order matches EX_SCHED: q7/k7 first (sims need them by
                # group 4-7), y chains mid, v chains last (needed g12+)
                proj_next, chains_next = None, held + deferred_y + held_v
            attention(wp, proj["qT"], proj["kT"], o2T, chains_next,
                      last=(wp + 1 == n_pair),
                      defer_trailing=(wp + 2 == n_pair))
            proj = proj_next


_CACHE = {}


def _build(n_win=N_WIN):
    key = n_win
    if key in _CACHE:
        return _CACHE[key]
    tok = n_win * WIN
    nc = bacc.Bacc(
        "TRN2", target_bir_lowering=False, debug=False, num_devices=N_CORES
    )
    # host-prepacked layouts: every DMA source is per-partition contiguous
    xqT = nc.dram_tensor("xqT", [128, tok * 8], BF16, kind="ExternalInput").ap()
    wq = nc.dram_tensor("Wq", [128, 8 * D], BF16, kind="ExternalInput").ap()
    wk = nc.dram_tensor("Wk", [128, 8 * D], BF16, kind="ExternalInput").ap()
    wv = nc.dram_tensor("Wv", [128, 8 * D], BF16, kind="ExternalInput").ap()
    wo = nc.dram_tensor("Wo", [128, 8 * D], BF16, kind="ExternalInput").ap()
    out = nc.dram_tensor("out", [tok, D], BF16, kind="ExternalOutput").ap()
    with tile.TileContext(nc) as tc:
        _body(tc, xqT, wq, wk, wv, wo, out, n_win)
    nc.compile()
    nc.m = get_hw_module(nc.m)
    _CACHE[key] = nc
    return nc


def _pack_w(W):
    # wpre[p, hf*4096 + kt*1024 + c] = W[hf*512 + kt*128 + p, c]
    bf = ml_dtypes.bfloat16
    W = np.asarray(W, np.float32).astype(bf)
    return np.ascontiguousarray(
        W.reshape(2, 4, 128, D).transpose(2, 0, 1, 3).reshape(128, 8 * D)
    )


def _wo_perm():
    # o2T index of head h=2m+s, dim d is (s*4 + m//2)*128 + 64*(m%2) + d;
    # permute Wo rows so the y matmul contracts o2T directly
    perm = np.empty(D, np.int64)
    for h in range(H):
        m, s = h // 2, h % 2
        base = (s * 4 + m // 2) * 128 + 64 * (m % 2)
        perm[base:base + DH] = np.arange(h * DH, (h + 1) * DH)
    return perm


def _pack_x(q2, n_pair):
    # xpre[p, (wp*2+hf)*2048 + dt*512 + t] = X[wp*512 + t, hf*512 + dt*128 + p]
    return np.ascontiguousarray(
        q2.reshape(n_pair, 512, 2, 4, 128)
        .transpose(4, 0, 2, 3, 1)
        .reshape(128, n_pair * 4096)
    )


def run(query, Wq, Wk, Wv, Wo, bo, n_win=N_WIN, **spmd_kwargs):
    nc = _build(n_win)
    tok = n_win * WIN
    n_pair = n_win // 2
    bf = ml_dtypes.bfloat16
    q2 = np.asarray(query, dtype=np.float32).reshape(-1, D).astype(bf)
    weights = {
        "Wq": _pack_w(Wq),
        "Wk": _pack_w(Wk),
        "Wv": _pack_w(Wv),
        "Wo": _pack_w(np.asarray(Wo, np.float32)[_wo_perm()]),
    }
    in_maps = []
    for c in range(N_CORES):
        m = {"xqT": _pack_x(q2[c * TOK:c * TOK + tok], n_pair)}
        m.update(weights)
        in_maps.append(m)
    try:
        res = bass_utils.run_bass_kernel_spmd(
            nc, in_maps, core_ids=list(range(N_CORES)), **spmd_kwargs
        )
    except Exception:
        # transient NRT_EXEC_UNIT_UNRECOVERABLE wedges clear on retry
        res = bass_utils.run_bass_kernel_spmd(
            nc, in_maps, core_ids=list(range(N_CORES)), **spmd_kwargs
        )
    outs = [np.asarray(res.results[c]["out"]).astype(np.float32)
            for c in range(N_CORES)]
    return outs, res


def kernel(query, context, Wq, Wk, Wv, Wo, bo):
    outs, _ = run(query, Wq, Wk, Wv, Wo, bo)
    y = np.concatenate(outs, axis=0).reshape(B, N, D)
    bo = np.asarray(bo, np.float32)
    if bo.any():
        y = y + bo  # bias is structurally zero for this problem; host-add keeps exactness
    return y.astype(np.float32)

